# revision 11
# baseline (speedup 1.0000x reference)
"""SimCLR contrastive loss on 8 TRN2 NeuronCores — v2.

Row-shard the N=8192 anchors across 8 cores (1024 each). Per core the
O(N^2) work is two 1024x8192 similarity blocks (anchor-anchor "pp" and
anchor-positive "pq"), each needing exp(2*s) row sums. v2 improvements
over the 145us baseline:

1) fp8e4 DoubleRow matmuls (0.5 cyc/row, 2x over f32r): inputs are
   host-normalized rows scaled by 16 and quantized to fp8e4m3, laid out
   [64, 2, N] (contraction = 64 partitions x 2 k-tiles).
2) The exp+row-sum bottleneck (ScalarE-only in the baseline) is split
   across THREE engines:
   - ACT: native exp with accum_out (exact path),
   - DVE: Schraudolph exp — tensor_scalar converts PSUM f32 s-values to
     int16 bf16-bit-codes (round-to-nearest, verified on HW), a second
     4x-mode tensor_scalar sums the bitcast bf16 values,
   - Pool: partition_all_reduce column sums of the Schraudolph codes.
3) pp symmetry: each core computes only 5 of 8 column blocks of its pp
   row-block (rolled cols [0, 5*1024)). Column sums of blocks 1..3
   (on Pool) provide the missing d in {5,6,7} partner contributions:
   for anchor i in core a, row sums cover j in cores a..a+4 and partner
   col sums cover j in cores a+5..a+7 — exact, no double count.

Host finish: combine ACT partials (exact), DVE/Pool partials (/KS
Schraudolph calibration), subtract the pp diagonal (recomputed exactly
from the fp8 values), loss_i = log(neg_i) - 2*s_i, mean.
"""

import numpy as np

N = 8192
D = 128
P = 128
NCORES = 8
M_LOCAL = N // NCORES          # 1024 own rows per core
T_OWN = M_LOCAL // P           # 8 own row chunks
TILE = 1024                    # consumer tile (cols); 2 PSUM banks f32
PP_BLOCKS = 5                  # pp col blocks computed (of 8)
SYM_T = (1, 2, 3)              # pp blocks whose col sums feed partners
PQ_BLOCKS = 6                  # pq col blocks row-summed (of 8)
PQT_T = (1, 2)                 # transposed-pq anchor blocks (col sums only)

EPS = 1e-8
KAPPA = 16.0                   # fp8 pre-scale; PSUM s' = 256*s
ACT_SCALE = 2.0 / (KAPPA * KAPPA)   # exp(2s) from PSUM value
# Schraudolph (bf16 codes, RNE convert — verified on HW):
#   i16 = rne(A1*psum + B1); bf16bits(i16) ~ exp(2s) * KS
A1 = float(np.float32(2.0 * (128.0 / np.log(2.0)) / 256.0))
B1 = 16250.0                   # 127*128 - 6
KS = 1.000910                  # E[schraudolph/exp], calibrated for B1

_CACHE = {}


def _schedule():
    """Static tile schedule: list of (mat, m, t, stream, csum).
    mat: 0 = pp (lhsT=zp, rhs=zp), 1 = pq (lhsT=zp, rhs=zq),
    2 = pq-transposed (lhsT=zq, rhs=zp; col sums only, no row partial).
    stream 'A' = ACT exp path, 'D' = DVE Schraudolph path.
    Greedy engine balance using the TimelineSim per-instr cost model."""
    act_c = 0.8333 * TILE + 419.0
    dve1 = 1.0417 * TILE + 170.0
    dve2 = 0.26 * TILE + 105.0
    clocks = {"A": 0.0, "D": 0.0}
    sched = []
    for m in range(T_OWN):
        # interleave csum tiles (pp sym, pqT) with plain tiles so the Pool
        # engine sees a steady stream instead of bursts
        window = ([(0, t) for t in range(PP_BLOCKS)]
                  + [(1, 0), (1, 1), (1, 2), (2, PQT_T[0]),
                     (1, 3), (1, 4), (1, 5), (2, PQT_T[1])])
        order = [window[i] for i in (0, 1, 5, 2, 6, 3, 7, 8, 12, 4, 9, 10, 11)]
        for mat, t in order:
                csum = (mat == 0 and t in SYM_T) or mat == 2
                if mat == 2:
                    st = "D"                  # instr1 only; Pool col-sums
                    clocks["D"] += dve1
                elif csum:
                    st = "D"
                    clocks["D"] += dve1 + dve2
                elif mat == 0 and t == 0:
                    st = "A"   # contains the pp diagonal — keep exact
                    clocks["A"] += act_c
                else:
                    st = ("A" if clocks["A"] + act_c <= clocks["D"]
                          + dve1 + dve2 else "D")
                    clocks[st] += act_c if st == "A" else dve1 + dve2
                sched.append((mat, m, t, st, csum))
    return sched


SCHED = _schedule()
N_ACT = sum(1 for e in SCHED if e[3] == "A")
N_DVE = sum(1 for e in SCHED if e[3] == "D" and e[0] != 2)
N_SYM = sum(1 for e in SCHED if e[4])


def _build_nc():
    import concourse.mybir as mybir
    import concourse.bass_isa as bass_isa
    from concourse import bacc
    from concourse.tile import TileContext
    from contextlib import ExitStack

    f32 = mybir.dt.float32
    bf16 = mybir.dt.bfloat16
    i16 = mybir.dt.int16
    fp8 = mybir.dt.float8e4
    AF = mybir.ActivationFunctionType
    ALU = mybir.AluOpType
    DR = mybir.MatmulPerfMode.DoubleRow

    nc = bacc.Bacc()
    zp_d = nc.dram_tensor("zpt", [64, 2, N], fp8, kind="ExternalInput")
    zq_d = nc.dram_tensor("zqt", [64, 2, N], fp8, kind="ExternalInput")
    outa_d = nc.dram_tensor("outa", [P, max(N_ACT, 1)], f32,
                            kind="ExternalOutput")
    outd_d = nc.dram_tensor("outd", [P, max(N_DVE, 1)], f32,
                            kind="ExternalOutput")
    cs_d = nc.dram_tensor("cs", [N_SYM, TILE], f32, kind="ExternalOutput")

    with TileContext(nc) as tc:
        with ExitStack() as ctx:
            sbuf = ctx.enter_context(tc.tile_pool(name="sbuf", bufs=1))
            z3p = sbuf.tile([64, 2, N], fp8)
            z3q = sbuf.tile([64, 2, N], fp8)
            outa = sbuf.tile([P, max(N_ACT, 1)], f32)
            outd = sbuf.tile([P, max(N_DVE, 1)], f32)
            trash = sbuf.tile([P, 2 * TILE], bf16)

            # chunked input loads so compute can start early; first chunks
            # small so the first tiles' matmuls start ASAP
            bounds = [0, 1024, 2048, 4096, 6144, N]
            for lo, hi in zip(bounds[:-1], bounds[1:]):
                nc.sync.dma_start(out=z3p[:, :, lo:hi], in_=zp_d[:, :, lo:hi])
            for lo, hi in zip(bounds[:-1], bounds[1:]):
                nc.sync.dma_start(out=z3q[:, :, lo:hi], in_=zq_d[:, :, lo:hi])

            act_ps = ctx.enter_context(
                tc.tile_pool(name="act_ps", bufs=2, space="PSUM"))
            dve_ps = ctx.enter_context(
                tc.tile_pool(name="dve_ps", bufs=2, space="PSUM"))
            q_pool = ctx.enter_context(tc.tile_pool(name="q_pool", bufs=8))
            cs_pool = ctx.enter_context(tc.tile_pool(name="cs_pool", bufs=4))

            ia = idv = isym = 0
            pend = None   # (qtile, ncols) awaiting a paired instr2
            last_m = -1
            for (mat, m, t, st, csum) in SCHED:
                if m != last_m and pend is not None:
                    qt2, nc2 = pend
                    nc.vector.tensor_scalar(
                        trash[:, 0:nc2], qt2[:, 0:nc2].bitcast(bf16), 1.0,
                        0.0, ALU.mult, ALU.add,
                        accum_out=outd[:, idv:idv + 1])
                    idv += 1
                    pend = None
                last_m = m
                zr = z3q if mat == 1 else z3p
                zl = z3q if mat == 2 else z3p
                lhsT = zl[:, :, m * P:(m + 1) * P]
                pool = act_ps if st == "A" else dve_ps
                pt = pool.tile([P, TILE], f32, tag="a" if st == "A" else "d")
                for j in range(TILE // 256):
                    c0 = t * TILE + j * 256
                    nc.tensor.matmul(
                        pt[:, j * 256:(j + 1) * 256],
                        lhsT=lhsT, rhs=zr[:, :, c0:c0 + 256],
                        start=True, stop=True, perf_mode=DR)
                if st == "A":
                    nc.scalar.activation(
                        pt[:, :], pt[:, :], AF.Exp, scale=ACT_SCALE,
                        accum_out=outa[:, ia:ia + 1])
                    ia += 1
                else:
                    if mat != 2 and pend is not None:
                        qt2, nc2 = pend
                        qs = qt2[:, nc2:nc2 + TILE]
                        nc.vector.tensor_scalar(qs, pt[:, :], A1, B1,
                                                ALU.mult, ALU.add)
                        nc.vector.tensor_scalar(
                            trash[:, 0:2 * TILE], qt2[:, :].bitcast(bf16),
                            1.0, 0.0, ALU.mult, ALU.add,
                            accum_out=outd[:, idv:idv + 1])
                        idv += 1
                        qcs = qs
                        pend = None
                    else:
                        qt = q_pool.tile([P, 2 * TILE], i16, tag="q")
                        nc.vector.tensor_scalar(qt[:, 0:TILE], pt[:, :],
                                                A1, B1, ALU.mult, ALU.add)
                        qcs = qt[:, 0:TILE]
                        if mat != 2:
                            pend = (qt, TILE)
                    if csum:
                        cst = cs_pool.tile([P, TILE], f32, tag="cs")
                        nc.gpsimd.partition_all_reduce(
                            cst[:, :], qcs.bitcast(bf16), 128,
                            bass_isa.ReduceOp.add)
                        nc.sync.dma_start(out=cs_d[isym:isym + 1, :],
                                          in_=cst[0:1, :])
                        isym += 1
            if pend is not None:
                qt2, nc2 = pend
                nc.vector.tensor_scalar(
                    trash[:, 0:nc2], qt2[:, 0:nc2].bitcast(bf16), 1.0, 0.0,
                    ALU.mult, ALU.add, accum_out=outd[:, idv:idv + 1])
                idv += 1

            nc.sync.dma_start(out=outa_d[:, :], in_=outa[:, :])
            nc.sync.dma_start(out=outd_d[:, :], in_=outd[:, :])

    nc.finalize()
    return nc


def _get_nc():
    if "nc" not in _CACHE:
        _CACHE["nc"] = _build_nc()
    return _CACHE["nc"]


def _host_prep(pred, positive):
    import ml_dtypes

    def nrm(x):
        n = np.sqrt(np.sum(x * x, axis=1, keepdims=True))
        return x / np.maximum(n, np.float32(EPS))

    zp = nrm(pred)
    zq = nrm(positive)
    s = np.sum(zp.astype(np.float64) * zq.astype(np.float64), axis=1)
    zp8 = (zp.T * np.float32(KAPPA)).astype(ml_dtypes.float8_e4m3)  # [D, N]
    zq8 = (zq.T * np.float32(KAPPA)).astype(ml_dtypes.float8_e4m3)
    # device-exact pp diagonal: sum_d fp8(16 zp)^2 / 256, exp(2s_ii~)
    dd = np.sum(zp8.astype(np.float64) ** 2, axis=0)
    diag_exp = np.exp(dd * ACT_SCALE)
    return zp8, zq8, s, diag_exp


LAST_RESULTS = None


def kernel(pred: np.ndarray, positive: np.ndarray) -> np.ndarray:
    global LAST_RESULTS
    import sys
    if "/opt/trn_rl_repo" not in sys.path:
        sys.path.insert(0, "/opt/trn_rl_repo")
    from concourse.bass_utils import run_bass_kernel_spmd

    pred = np.ascontiguousarray(np.asarray(pred, dtype=np.float32))
    positive = np.ascontiguousarray(np.asarray(positive, dtype=np.float32))

    zp8, zq8, s, diag_exp = _host_prep(pred, positive)

    def roll3(z8, k):
        r = np.concatenate([z8[:, k:], z8[:, :k]], axis=1)      # [128, N]
        return np.ascontiguousarray(r.reshape(2, 64, N).transpose(1, 0, 2))

    nc = _get_nc()
    in_maps = []
    for c in range(NCORES):
        k = c * M_LOCAL
        in_maps.append({"zpt": roll3(zp8, k), "zqt": roll3(zq8, k)})
    res = run_bass_kernel_spmd(nc, in_maps, core_ids=list(range(NCORES)))
    LAST_RESULTS = res

    neg = np.zeros(N, dtype=np.float64)
    inv_ks = 1.0 / KS
    for c in range(NCORES):
        oa = np.asarray(res.results[c]["outa"], dtype=np.float64)
        od = np.asarray(res.results[c]["outd"], dtype=np.float64)
        cs = np.asarray(res.results[c]["cs"], dtype=np.float64)
        ia = idv = isym = 0
        dpend = 0
        last_m = -1
        for (mat, m, t, st, csum) in SCHED:
            rows = c * M_LOCAL + m * P + np.arange(P)
            if m != last_m and dpend:
                prows = c * M_LOCAL + last_m * P + np.arange(P)
                neg[prows] += od[:, idv] * inv_ks
                idv += 1
                dpend = 0
            last_m = m
            if st == "A":
                neg[rows] += oa[:, ia]
                ia += 1
            else:
                if mat != 2:
                    if dpend:
                        neg[rows] += od[:, idv] * inv_ks
                        idv += 1
                        dpend = 0
                    else:
                        dpend = 1
                if csum:
                    anchors = (c * M_LOCAL + t * TILE + np.arange(TILE)) % N
                    neg[anchors] += cs[isym, :] * inv_ks
                    isym += 1
        if dpend:
            prows = c * M_LOCAL + last_m * P + np.arange(P)
            neg[prows] += od[:, idv] * inv_ks
            idv += 1
    neg -= diag_exp
    loss = np.mean(np.log(neg) - 2.0 * s)
    return np.float32(loss)


# revision 16
# speedup vs baseline: 1.0023x; 1.0023x over previous
"""SimCLR contrastive loss on 8 TRN2 NeuronCores — v2.

Row-shard the N=8192 anchors across 8 cores (1024 each). Per core the
O(N^2) work is two 1024x8192 similarity blocks (anchor-anchor "pp" and
anchor-positive "pq"), each needing exp(2*s) row sums. v2 improvements
over the 145us baseline:

1) fp8e4 DoubleRow matmuls (0.5 cyc/row, 2x over f32r): inputs are
   host-normalized rows scaled by 16 and quantized to fp8e4m3, laid out
   [64, 2, N] (contraction = 64 partitions x 2 k-tiles).
2) The exp+row-sum bottleneck (ScalarE-only in the baseline) is split
   across THREE engines:
   - ACT: native exp with accum_out (exact path),
   - DVE: Schraudolph exp — tensor_scalar converts PSUM f32 s-values to
     int16 bf16-bit-codes (round-to-nearest, verified on HW), a second
     4x-mode tensor_scalar sums the bitcast bf16 values,
   - Pool: partition_all_reduce column sums of the Schraudolph codes.
3) pp symmetry: each core computes only 5 of 8 column blocks of its pp
   row-block (rolled cols [0, 5*1024)). Column sums of blocks 1..3
   (on Pool) provide the missing d in {5,6,7} partner contributions:
   for anchor i in core a, row sums cover j in cores a..a+4 and partner
   col sums cover j in cores a+5..a+7 — exact, no double count.

Host finish: combine ACT partials (exact), DVE/Pool partials (/KS
Schraudolph calibration), subtract the pp diagonal (recomputed exactly
from the fp8 values), loss_i = log(neg_i) - 2*s_i, mean.
"""

import numpy as np

N = 8192
D = 128
P = 128
NCORES = 8
M_LOCAL = N // NCORES          # 1024 own rows per core
T_OWN = M_LOCAL // P           # 8 own row chunks
TILE = 1024                    # consumer tile (cols); 2 PSUM banks f32
PP_BLOCKS = 5                  # pp col blocks computed (of 8)
SYM_T = (1, 2, 3)              # pp blocks whose col sums feed partners
PQ_BLOCKS = 6                  # pq col blocks row-summed (of 8)
PQT_T = (1, 2)                 # transposed-pq anchor blocks (col sums only)

EPS = 1e-8
KAPPA = 16.0                   # fp8 pre-scale; PSUM s' = 256*s
ACT_SCALE = 2.0 / (KAPPA * KAPPA)   # exp(2s) from PSUM value
# Schraudolph (bf16 codes, RNE convert — verified on HW):
#   i16 = rne(A1*psum + B1); bf16bits(i16) ~ exp(2s) * KS
A1 = float(np.float32(2.0 * (128.0 / np.log(2.0)) / 256.0))
B1 = 16250.0                   # 127*128 - 6
KS = 1.000910                  # E[schraudolph/exp], calibrated for B1

_CACHE = {}


def _schedule():
    """Static tile schedule: list of (mat, m, t, stream, csum).
    mat: 0 = pp (lhsT=zp, rhs=zp), 1 = pq (lhsT=zp, rhs=zq),
    2 = pq-transposed (lhsT=zq, rhs=zp; col sums only, no row partial).
    stream 'A' = ACT exp path, 'D' = DVE Schraudolph path.
    Greedy engine balance using the TimelineSim per-instr cost model."""
    act_c = 0.8333 * TILE + 419.0
    dve1 = 1.0417 * TILE + 170.0
    dve2 = 0.26 * TILE + 105.0
    clocks = {"A": 0.0, "D": 0.0}
    sched = []
    for m in range(T_OWN):
        # interleave csum tiles (pp sym, pqT) with plain tiles so the Pool
        # engine sees a steady stream instead of bursts
        window = ([(0, t) for t in range(PP_BLOCKS)]
                  + [(1, 0), (1, 1), (1, 2), (2, PQT_T[0]),
                     (1, 3), (1, 4), (1, 5), (2, PQT_T[1])])
        idx = ((0, 1, 2, 3, 4, 5, 6, 7, 8, 12, 9, 10, 11) if m == 0 else
               (0, 1, 5, 2, 6, 3, 7, 8, 12, 4, 9, 10, 11))
        order = [window[i] for i in idx]
        for mat, t in order:
                csum = (mat == 0 and t in SYM_T) or mat == 2
                if mat == 2:
                    st = "D"                  # instr1 only; Pool col-sums
                    clocks["D"] += dve1
                elif csum:
                    st = "D"
                    clocks["D"] += dve1 + dve2
                elif mat == 0 and t == 0:
                    st = "A"   # contains the pp diagonal — keep exact
                    clocks["A"] += act_c
                else:
                    st = ("A" if clocks["A"] + act_c <= clocks["D"]
                          + dve1 + dve2 else "D")
                    clocks[st] += act_c if st == "A" else dve1 + dve2
                sched.append((mat, m, t, st, csum))
    return sched


SCHED = _schedule()
N_ACT = sum(1 for e in SCHED if e[3] == "A")
N_DVE = sum(1 for e in SCHED if e[3] == "D" and e[0] != 2)
N_SYM = sum(1 for e in SCHED if e[4])


def _build_nc():
    import concourse.mybir as mybir
    import concourse.bass_isa as bass_isa
    from concourse import bacc
    from concourse.tile import TileContext
    from contextlib import ExitStack

    f32 = mybir.dt.float32
    bf16 = mybir.dt.bfloat16
    i16 = mybir.dt.int16
    fp8 = mybir.dt.float8e4
    AF = mybir.ActivationFunctionType
    ALU = mybir.AluOpType
    DR = mybir.MatmulPerfMode.DoubleRow

    nc = bacc.Bacc()
    zp_d = nc.dram_tensor("zpt", [64, 2, N], fp8, kind="ExternalInput")
    zq_d = nc.dram_tensor("zqt", [64, 2, N], fp8, kind="ExternalInput")
    outa_d = nc.dram_tensor("outa", [P, max(N_ACT, 1)], f32,
                            kind="ExternalOutput")
    outd_d = nc.dram_tensor("outd", [P, max(N_DVE, 1)], f32,
                            kind="ExternalOutput")
    cs_d = nc.dram_tensor("cs", [N_SYM, TILE], f32, kind="ExternalOutput")

    with TileContext(nc) as tc:
        with ExitStack() as ctx:
            sbuf = ctx.enter_context(tc.tile_pool(name="sbuf", bufs=1))
            z3p = sbuf.tile([64, 2, N], fp8)
            z3q = sbuf.tile([64, 2, N], fp8)
            outa = sbuf.tile([P, max(N_ACT, 1)], f32)
            outd = sbuf.tile([P, max(N_DVE, 1)], f32)
            trash = sbuf.tile([P, 2 * TILE], bf16)

            # chunked input loads so compute can start early; first chunks
            # small so the first tiles' matmuls start ASAP
            bounds = [0, 1024, 2048, 4096, 6144, N]
            for lo, hi in zip(bounds[:-1], bounds[1:]):
                nc.sync.dma_start(out=z3p[:, :, lo:hi], in_=zp_d[:, :, lo:hi])
            for lo, hi in zip(bounds[:-1], bounds[1:]):
                nc.sync.dma_start(out=z3q[:, :, lo:hi], in_=zq_d[:, :, lo:hi])

            act_ps = ctx.enter_context(
                tc.tile_pool(name="act_ps", bufs=2, space="PSUM"))
            dve_ps = ctx.enter_context(
                tc.tile_pool(name="dve_ps", bufs=2, space="PSUM"))
            q_pool = ctx.enter_context(tc.tile_pool(name="q_pool", bufs=8))
            cs_pool = ctx.enter_context(tc.tile_pool(name="cs_pool", bufs=4))

            ia = idv = isym = 0
            pend = None   # (qtile, ncols) awaiting a paired instr2
            last_m = -1
            for (mat, m, t, st, csum) in SCHED:
                if m != last_m and pend is not None:
                    qt2, nc2 = pend
                    nc.vector.tensor_scalar(
                        trash[:, 0:nc2], qt2[:, 0:nc2].bitcast(bf16), 1.0,
                        0.0, ALU.mult, ALU.add,
                        accum_out=outd[:, idv:idv + 1])
                    idv += 1
                    pend = None
                last_m = m
                zr = z3q if mat == 1 else z3p
                zl = z3q if mat == 2 else z3p
                lhsT = zl[:, :, m * P:(m + 1) * P]
                pool = act_ps if st == "A" else dve_ps
                pt = pool.tile([P, TILE], f32, tag="a" if st == "A" else "d")
                for j in range(TILE // 256):
                    c0 = t * TILE + j * 256
                    nc.tensor.matmul(
                        pt[:, j * 256:(j + 1) * 256],
                        lhsT=lhsT, rhs=zr[:, :, c0:c0 + 256],
                        start=True, stop=True, perf_mode=DR)
                if st == "A":
                    nc.scalar.activation(
                        pt[:, :], pt[:, :], AF.Exp, scale=ACT_SCALE,
                        accum_out=outa[:, ia:ia + 1])
                    ia += 1
                else:
                    if mat != 2 and pend is not None:
                        qt2, nc2 = pend
                        qs = qt2[:, nc2:nc2 + TILE]
                        nc.vector.tensor_scalar(qs, pt[:, :], A1, B1,
                                                ALU.mult, ALU.add)
                        nc.vector.tensor_scalar(
                            trash[:, 0:2 * TILE], qt2[:, :].bitcast(bf16),
                            1.0, 0.0, ALU.mult, ALU.add,
                            accum_out=outd[:, idv:idv + 1])
                        idv += 1
                        qcs = qs
                        pend = None
                    else:
                        qt = q_pool.tile([P, 2 * TILE], i16, tag="q")
                        nc.vector.tensor_scalar(qt[:, 0:TILE], pt[:, :],
                                                A1, B1, ALU.mult, ALU.add)
                        qcs = qt[:, 0:TILE]
                        if mat != 2:
                            pend = (qt, TILE)
                    if csum:
                        cst = cs_pool.tile([P, TILE], f32, tag="cs")
                        nc.gpsimd.partition_all_reduce(
                            cst[:, :], qcs.bitcast(bf16), 128,
                            bass_isa.ReduceOp.add)
                        nc.sync.dma_start(out=cs_d[isym:isym + 1, :],
                                          in_=cst[0:1, :])
                        isym += 1
            if pend is not None:
                qt2, nc2 = pend
                nc.vector.tensor_scalar(
                    trash[:, 0:nc2], qt2[:, 0:nc2].bitcast(bf16), 1.0, 0.0,
                    ALU.mult, ALU.add, accum_out=outd[:, idv:idv + 1])
                idv += 1

            nc.sync.dma_start(out=outa_d[:, :], in_=outa[:, :])
            nc.sync.dma_start(out=outd_d[:, :], in_=outd[:, :])

    nc.finalize()
    return nc


def _get_nc():
    if "nc" not in _CACHE:
        _CACHE["nc"] = _build_nc()
    return _CACHE["nc"]


def _host_prep(pred, positive):
    import ml_dtypes

    def nrm(x):
        n = np.sqrt(np.sum(x * x, axis=1, keepdims=True))
        return x / np.maximum(n, np.float32(EPS))

    zp = nrm(pred)
    zq = nrm(positive)
    s = np.sum(zp.astype(np.float64) * zq.astype(np.float64), axis=1)
    zp8 = (zp.T * np.float32(KAPPA)).astype(ml_dtypes.float8_e4m3)  # [D, N]
    zq8 = (zq.T * np.float32(KAPPA)).astype(ml_dtypes.float8_e4m3)
    # device-exact pp diagonal: sum_d fp8(16 zp)^2 / 256, exp(2s_ii~)
    dd = np.sum(zp8.astype(np.float64) ** 2, axis=0)
    diag_exp = np.exp(dd * ACT_SCALE)
    return zp8, zq8, s, diag_exp


LAST_RESULTS = None


def kernel(pred: np.ndarray, positive: np.ndarray) -> np.ndarray:
    global LAST_RESULTS
    import sys
    if "/opt/trn_rl_repo" not in sys.path:
        sys.path.insert(0, "/opt/trn_rl_repo")
    from concourse.bass_utils import run_bass_kernel_spmd

    pred = np.ascontiguousarray(np.asarray(pred, dtype=np.float32))
    positive = np.ascontiguousarray(np.asarray(positive, dtype=np.float32))

    zp8, zq8, s, diag_exp = _host_prep(pred, positive)

    def roll3(z8, k):
        r = np.concatenate([z8[:, k:], z8[:, :k]], axis=1)      # [128, N]
        return np.ascontiguousarray(r.reshape(2, 64, N).transpose(1, 0, 2))

    nc = _get_nc()
    in_maps = []
    for c in range(NCORES):
        k = c * M_LOCAL
        in_maps.append({"zpt": roll3(zp8, k), "zqt": roll3(zq8, k)})
    res = run_bass_kernel_spmd(nc, in_maps, core_ids=list(range(NCORES)))
    LAST_RESULTS = res

    neg = np.zeros(N, dtype=np.float64)
    inv_ks = 1.0 / KS
    for c in range(NCORES):
        oa = np.asarray(res.results[c]["outa"], dtype=np.float64)
        od = np.asarray(res.results[c]["outd"], dtype=np.float64)
        cs = np.asarray(res.results[c]["cs"], dtype=np.float64)
        ia = idv = isym = 0
        dpend = 0
        last_m = -1
        for (mat, m, t, st, csum) in SCHED:
            rows = c * M_LOCAL + m * P + np.arange(P)
            if m != last_m and dpend:
                prows = c * M_LOCAL + last_m * P + np.arange(P)
                neg[prows] += od[:, idv] * inv_ks
                idv += 1
                dpend = 0
            last_m = m
            if st == "A":
                neg[rows] += oa[:, ia]
                ia += 1
            else:
                if mat != 2:
                    if dpend:
                        neg[rows] += od[:, idv] * inv_ks
                        idv += 1
                        dpend = 0
                    else:
                        dpend = 1
                if csum:
                    anchors = (c * M_LOCAL + t * TILE + np.arange(TILE)) % N
                    neg[anchors] += cs[isym, :] * inv_ks
                    isym += 1
        if dpend:
            prows = c * M_LOCAL + last_m * P + np.arange(P)
            neg[prows] += od[:, idv] * inv_ks
            idv += 1
    neg -= diag_exp
    loss = np.mean(np.log(neg) - 2.0 * s)
    return np.float32(loss)
